# revision 9
# baseline (speedup 1.0000x reference)
"""Trainium2 Bass kernel for JoinAndSubsample (strided window gather).

reference semantics: x[B,T,D] -> edge-pad time by (3,3) -> out[B,TOUT,7*D]
where out[b,t,:] = concat(xp[b, 3t .. 3t+6, :]).  Since the 7 window frames
are consecutive, each output row is a contiguous 7*D-float slice of the
padded input starting at frame 3t -> the whole op is a strided-DMA copy.

Strategy (per core, pure data parallel over batch, 4 batches/core):
  - SBUF staging, partition p = b*32 + c (batch-major): 128 partitions =
    4 batches x 32 time-chunks; each partition holds its chunk's input
    frames incl. 3-frame halos (262 frames * 80 f32 = 83,840 B).
  - Replicate-padding loaded straight from HBM (frame 0 / frame T-1),
    no SBUF->SBUF wave.
  - Stores: overlapping-window DMA reads from SBUF (src stride 960 B,
    elem 2240 B) to contiguous DRAM output.
  - mode="overlap": loads issue on the SP (sync) HWDGE ring with
    per-batch semaphores; stores issue on the ACT (scalar) ring chasing
    those semaphores, so the SDMA engines interleave load and store
    packets and the two HBM streams overlap.
  - mode="serial": everything on the sync ring with one barrier.
  HBM traffic/core = 10.5 MB read + 24.5 MB write (minimum possible).
"""

from contextlib import ExitStack

import numpy as np

import concourse.bass as bass
import concourse.mybir as mybir
from concourse.ap import AP
from concourse.bass_utils import run_bass_kernel_spmd

LEFT, RIGHT, STRIDE, D = 3, 3, 3, 80
W = LEFT + RIGHT + 1            # 7 frames / window
B, T = 32, 8192
NCORES = 8
BPC = B // NCORES               # 4 batches per core
TOUT = (T - 1) // STRIDE + 1    # 2731
NCHUNK = 32                     # time-chunks per batch; BPC*NCHUNK = 128
MODE = "overlap"


def build_nc(bpc=BPC, t=T, d=D, left=LEFT, right=RIGHT, stride=STRIDE,
             nchunk=NCHUNK, mode=MODE, sim_init=False, reps=1):
    """Build the per-core Bass module (parametric for small-scale sim tests).

    reps>1 repeats the whole load+store sequence serially (cumulative
    semaphore targets) — used only for marginal-time benchmarking."""
    w = left + right + 1
    tout = (t - 1) // stride + 1
    nt = -(-tout // nchunk)                 # output rows per chunk (ceil)
    nt_last = tout - nt * (nchunk - 1)      # rows in last chunk
    assert nt_last >= 1
    fpc = stride * nt + (w - stride)        # frames per partition incl halo
    fpc_last = stride * nt_last + (w - stride)
    free = fpc * d                          # f32 elems per partition
    od = w * d                              # output row elems
    cl = nchunk - 1                         # last chunk index
    cl_start = cl * nt * stride - left      # first input frame of last chunk
    cl_cnt = t - cl_start                   # real frames available
    assert 0 < cl_cnt <= fpc_last
    n_rpad = fpc_last - cl_cnt              # right-pad frames to replicate
    # bulk load covers chunks 1..nchunk-2 entirely inside [0, t)
    assert (cl - 1) * nt * stride - left + fpc <= t
    assert nt * stride - left >= 0
    assert bpc * nchunk <= 128

    # race detector is tensor-granular for DMA writes; our concurrent DMAs
    # write disjoint partitions/slots, so disable it (sim-only effect).
    nc = bass.Bass(detect_race_conditions=False)
    x = nc.declare_dram_parameter("x", [bpc, t, d], mybir.dt.float32,
                                  isOutput=False)
    y = nc.declare_dram_parameter("y", [bpc, tout, od], mybir.dt.float32,
                                  isOutput=True)

    with ExitStack() as ctx:
        tile = ctx.enter_context(
            nc.sbuf_tensor([bpc * nchunk, free], mybir.dt.float32))
        psem = ctx.enter_context(nc.semaphore("pad_sem"))
        bsem = [ctx.enter_context(nc.semaphore(f"b{b}_sem"))
                for b in range(bpc)]
        ssem = ctx.enter_context(nc.semaphore("store_sem"))
        isem = ctx.enter_context(nc.semaphore("init_sem"))
        block = ctx.enter_context(nc.Block())

        sb = tile[:].tensor
        n_pads = left + n_rpad

        if sim_init:
            # CoreSim's shadow-init tracker can't follow partition-strided
            # DMA writes; pre-memset the tile so full-tile reads validate.
            @block.vector
            def _(vector):
                vector.memset(tile[:], 0.0).then_inc(isem, 1)

        def issue_loads(eng, r=0, wait_stores=True):
            if sim_init and r == 0:
                eng.wait_ge(isem, 1)
            if r > 0 and wait_stores:
                # WAR: rep r's loads overwrite SBUF read by rep r-1's stores
                eng.wait_ge(ssem, (bpc + 1) * 16 * r)
            # pads straight from HBM: left = frame 0, right = frame t-1,
            # each DMA covers all batches (partition stride nchunk)
            for k in range(left):
                eng.dma_start(
                    out=AP(sb, k * d, [[nchunk * free, bpc], [1, d]]),
                    in_=AP(x, 0, [[t * d, bpc], [1, d]]),
                ).then_inc(psem, 16)
            for j in range(n_rpad):
                eng.dma_start(
                    out=AP(sb, cl * free + (cl_cnt + j) * d,
                           [[nchunk * free, bpc], [1, d]]),
                    in_=AP(x, (t - 1) * d, [[t * d, bpc], [1, d]]),
                ).then_inc(psem, 16)
            for b in range(bpc):
                # bulk: chunks 1..nchunk-2, frames [nt*3*c - 3, +fpc)
                eng.dma_start(
                    out=AP(sb, (b * nchunk + 1) * free,
                           [[free, nchunk - 2], [1, free]]),
                    in_=AP(x, b * t * d + (nt * stride - left) * d,
                           [[nt * stride * d, nchunk - 2], [1, free]]),
                ).then_inc(bsem[b], 16)
                # chunk 0: frames [0, fpc-left) land at slot `left`
                eng.dma_start(
                    out=AP(sb, b * nchunk * free + left * d,
                           [[free, 1], [1, (fpc - left) * d]]),
                    in_=AP(x, b * t * d, [[t * d, 1], [1, (fpc - left) * d]]),
                ).then_inc(bsem[b], 16)
                # last chunk: frames [cl_start, t) land at slot 0
                eng.dma_start(
                    out=AP(sb, (b * nchunk + cl) * free,
                           [[free, 1], [1, cl_cnt * d]]),
                    in_=AP(x, b * t * d + cl_start * d,
                           [[t * d, 1], [1, cl_cnt * d]]),
                ).then_inc(bsem[b], 16)

        def issue_stores(eng, r=0):
            eng.wait_ge(psem, n_pads * 16 * (r + 1))
            for b in range(bpc):
                eng.wait_ge(bsem[b], 3 * 16 * (r + 1))
                # chunks 0..nchunk-2 (nt rows each); dst rows contiguous
                eng.dma_start(
                    out=AP(y, b * tout * od, [[nt * od, cl], [1, nt * od]]),
                    in_=AP(sb, b * nchunk * free,
                           [[free, cl], [stride * d, nt], [1, od]]),
                ).then_inc(ssem, 16)
            # last chunk, all batches in one DMA
            eng.dma_start(
                out=AP(y, cl * nt * od, [[tout * od, bpc], [1, nt_last * od]]),
                in_=AP(sb, cl * free,
                       [[nchunk * free, bpc], [stride * d, nt_last], [1, od]]),
            ).then_inc(ssem, 16)
            eng.wait_ge(ssem, (bpc + 1) * 16 * (r + 1))

        if mode == "overlap":
            @block.sync
            def _(sync):
                for r in range(reps):
                    issue_loads(sync, r)

            @block.scalar
            def _(scalar):
                for r in range(reps):
                    issue_stores(scalar, r)
        elif mode == "serial":
            @block.sync
            def _(sync):
                for r in range(reps):
                    issue_loads(sync, r)
                    issue_stores(sync, r)
        elif mode == "loadonly":     # diagnostic: loads only, y stays 0
            @block.sync
            def _(sync):
                for r in range(reps):
                    if r > 0:      # serialize reps on load completion
                        sync.wait_ge(psem, n_pads * 16 * r)
                        for b in range(bpc):
                            sync.wait_ge(bsem[b], 3 * 16 * r)
                    issue_loads(sync, r, wait_stores=False)
                sync.wait_ge(psem, n_pads * 16 * reps)
                for b in range(bpc):
                    sync.wait_ge(bsem[b], 3 * 16 * reps)
        elif mode == "storeonly":    # diagnostic: stores of uninit SBUF
            @block.scalar
            def _(scalar):
                for r in range(reps):
                    for b in range(bpc):
                        scalar.dma_start(
                            out=AP(y, b * tout * od,
                                   [[nt * od, cl], [1, nt * od]]),
                            in_=AP(sb, b * nchunk * free,
                                   [[free, cl], [stride * d, nt], [1, od]]),
                        ).then_inc(ssem, 16)
                    scalar.dma_start(
                        out=AP(y, cl * nt * od,
                               [[tout * od, bpc], [1, nt_last * od]]),
                        in_=AP(sb, cl * free,
                               [[nchunk * free, bpc], [stride * d, nt_last],
                                [1, od]]),
                    ).then_inc(ssem, 16)
                    scalar.wait_ge(ssem, (bpc + 1) * 16 * (r + 1))
        else:
            raise ValueError(mode)

    return nc


_NC = None


def _get_nc():
    global _NC
    if _NC is None:
        _NC = build_nc()
    return _NC


def kernel(**inputs):
    x = np.ascontiguousarray(inputs["x"], dtype=np.float32)
    assert x.shape == (B, T, D)
    nc = _get_nc()
    in_maps = [{"x": x[i * BPC:(i + 1) * BPC]} for i in range(NCORES)]
    res = run_bass_kernel_spmd(nc, in_maps, list(range(NCORES)))
    return np.concatenate([res.results[i]["y"] for i in range(NCORES)], axis=0)


# revision 10
# speedup vs baseline: 661.4788x; 661.4788x over previous
"""Trainium2 Bass kernel for JoinAndSubsample (strided window gather).

reference semantics: x[B,T,D] -> edge-pad time by (3,3) -> out[B,TOUT,7*D]
where out[b,t,:] = concat(xp[b, 3t .. 3t+6, :]).  Since the 7 window frames
are consecutive, each output row is a contiguous 7*D-float slice of the
padded input starting at frame 3t -> the whole op is a strided-DMA copy.

Strategy (per core, pure data parallel over batch, 4 batches/core):
  - SBUF staging, partition p = b*32 + c (batch-major): 128 partitions =
    4 batches x 32 time-chunks; each partition holds its chunk's input
    frames incl. 3-frame halos (262 frames * 80 f32 = 83,840 B).
  - Replicate-padding loaded straight from HBM (frame 0 / frame T-1),
    no SBUF->SBUF wave.
  - Stores: overlapping-window DMA reads from SBUF (src stride 960 B,
    elem 2240 B) to contiguous DRAM output.
  - mode="overlap": loads issue on the SP (sync) HWDGE ring with
    per-batch semaphores; stores issue on the ACT (scalar) ring chasing
    those semaphores, so the SDMA engines interleave load and store
    packets and the two HBM streams overlap.
  - mode="serial": everything on the sync ring with one barrier.
  HBM traffic/core = 10.5 MB read + 24.5 MB write (minimum possible).
"""

from contextlib import ExitStack

import numpy as np

import concourse.bass as bass
import concourse.mybir as mybir
from concourse.ap import AP
from concourse.bass_utils import run_bass_kernel_spmd

LEFT, RIGHT, STRIDE, D = 3, 3, 3, 80
W = LEFT + RIGHT + 1            # 7 frames / window
B, T = 32, 8192
NCORES = 8
BPC = B // NCORES               # 4 batches per core
TOUT = (T - 1) // STRIDE + 1    # 2731
NCHUNK = 32                     # time-chunks per batch; BPC*NCHUNK = 128
MODE = "overlap"


def build_nc(bpc=BPC, t=T, d=D, left=LEFT, right=RIGHT, stride=STRIDE,
             nchunk=NCHUNK, mode=MODE, sim_init=False, reps=1):
    """Build the per-core Bass module (parametric for small-scale sim tests).

    reps>1 repeats the whole load+store sequence serially (cumulative
    semaphore targets) — used only for marginal-time benchmarking."""
    w = left + right + 1
    tout = (t - 1) // stride + 1
    nt = -(-tout // nchunk)                 # output rows per chunk (ceil)
    nt_last = tout - nt * (nchunk - 1)      # rows in last chunk
    assert nt_last >= 1
    fpc = stride * nt + (w - stride)        # frames per partition incl halo
    fpc_last = stride * nt_last + (w - stride)
    free = fpc * d                          # f32 elems per partition
    od = w * d                              # output row elems
    cl = nchunk - 1                         # last chunk index
    cl_start = cl * nt * stride - left      # first input frame of last chunk
    cl_cnt = t - cl_start                   # real frames available
    assert 0 < cl_cnt <= fpc_last
    n_rpad = fpc_last - cl_cnt              # right-pad frames to replicate
    # bulk load covers chunks 1..nchunk-2 entirely inside [0, t)
    assert (cl - 1) * nt * stride - left + fpc <= t
    assert nt * stride - left >= 0
    assert bpc * nchunk <= 128

    # race detector is tensor-granular for DMA writes; our concurrent DMAs
    # write disjoint partitions/slots, so disable it (sim-only effect).
    nc = bass.Bass(detect_race_conditions=False)
    x = nc.declare_dram_parameter("x", [bpc, t, d], mybir.dt.float32,
                                  isOutput=False)
    y = nc.declare_dram_parameter("y", [bpc, tout, od], mybir.dt.float32,
                                  isOutput=True)

    with ExitStack() as ctx:
        tile = ctx.enter_context(
            nc.sbuf_tensor([bpc * nchunk, free], mybir.dt.float32))
        psem = ctx.enter_context(nc.semaphore("pad_sem"))
        bsem = [ctx.enter_context(nc.semaphore(f"b{b}_sem"))
                for b in range(bpc)]
        ssem = ctx.enter_context(nc.semaphore("store_sem"))
        isem = ctx.enter_context(nc.semaphore("init_sem"))
        block = ctx.enter_context(nc.Block())

        sb = tile[:].tensor
        n_pads = left + n_rpad

        if sim_init:
            # CoreSim's shadow-init tracker can't follow partition-strided
            # DMA writes; pre-memset the tile so full-tile reads validate.
            @block.vector
            def _(vector):
                vector.memset(tile[:], 0.0).then_inc(isem, 1)

        def issue_loads(eng, r=0, wait_stores=True):
            if sim_init and r == 0:
                eng.wait_ge(isem, 1)
            if r > 0 and wait_stores:
                # WAR: rep r's loads overwrite SBUF read by rep r-1's stores
                eng.wait_ge(ssem, (bpc + 1) * 16 * r)
            # pads straight from HBM: left = frame 0, right = frame t-1,
            # each DMA covers all batches (partition stride nchunk)
            for k in range(left):
                eng.dma_start(
                    out=AP(sb, k * d, [[nchunk * free, bpc], [1, d]]),
                    in_=AP(x, 0, [[t * d, bpc], [1, d]]),
                ).then_inc(psem, 16)
            for j in range(n_rpad):
                eng.dma_start(
                    out=AP(sb, cl * free + (cl_cnt + j) * d,
                           [[nchunk * free, bpc], [1, d]]),
                    in_=AP(x, (t - 1) * d, [[t * d, bpc], [1, d]]),
                ).then_inc(psem, 16)
            for b in range(bpc):
                # bulk: chunks 1..nchunk-2, frames [nt*3*c - 3, +fpc)
                eng.dma_start(
                    out=AP(sb, (b * nchunk + 1) * free,
                           [[free, nchunk - 2], [1, free]]),
                    in_=AP(x, b * t * d + (nt * stride - left) * d,
                           [[nt * stride * d, nchunk - 2], [1, free]]),
                ).then_inc(bsem[b], 16)
                # chunk 0: frames [0, fpc-left) land at slot `left`
                eng.dma_start(
                    out=AP(sb, b * nchunk * free + left * d,
                           [[free, 1], [1, (fpc - left) * d]]),
                    in_=AP(x, b * t * d, [[t * d, 1], [1, (fpc - left) * d]]),
                ).then_inc(bsem[b], 16)
                # last chunk: frames [cl_start, t) land at slot 0
                eng.dma_start(
                    out=AP(sb, (b * nchunk + cl) * free,
                           [[free, 1], [1, cl_cnt * d]]),
                    in_=AP(x, b * t * d + cl_start * d,
                           [[t * d, 1], [1, cl_cnt * d]]),
                ).then_inc(bsem[b], 16)

        def issue_stores(eng, r=0):
            eng.wait_ge(psem, n_pads * 16 * (r + 1))
            for b in range(bpc):
                eng.wait_ge(bsem[b], 3 * 16 * (r + 1))
                # chunks 0..nchunk-2 (nt rows each); dst rows contiguous
                eng.dma_start(
                    out=AP(y, b * tout * od, [[nt * od, cl], [1, nt * od]]),
                    in_=AP(sb, b * nchunk * free,
                           [[free, cl], [stride * d, nt], [1, od]]),
                ).then_inc(ssem, 16)
            # last chunk, all batches in one DMA
            eng.dma_start(
                out=AP(y, cl * nt * od, [[tout * od, bpc], [1, nt_last * od]]),
                in_=AP(sb, cl * free,
                       [[nchunk * free, bpc], [stride * d, nt_last], [1, od]]),
            ).then_inc(ssem, 16)
            eng.wait_ge(ssem, (bpc + 1) * 16 * (r + 1))

        if mode == "overlap":
            @block.sync
            def _(sync):
                for r in range(reps):
                    issue_loads(sync, r)

            @block.scalar
            def _(scalar):
                for r in range(reps):
                    issue_stores(scalar, r)
        elif mode == "serial":
            @block.sync
            def _(sync):
                for r in range(reps):
                    issue_loads(sync, r)
                    issue_stores(sync, r)
        elif mode == "loadonly":     # diagnostic: loads only, y stays 0
            @block.sync
            def _(sync):
                for r in range(reps):
                    if r > 0:      # serialize reps on load completion
                        sync.wait_ge(psem, n_pads * 16 * r)
                        for b in range(bpc):
                            sync.wait_ge(bsem[b], 3 * 16 * r)
                    issue_loads(sync, r, wait_stores=False)
                sync.wait_ge(psem, n_pads * 16 * reps)
                for b in range(bpc):
                    sync.wait_ge(bsem[b], 3 * 16 * reps)
        elif mode == "storeonly":    # diagnostic: stores of uninit SBUF
            @block.scalar
            def _(scalar):
                for r in range(reps):
                    for b in range(bpc):
                        scalar.dma_start(
                            out=AP(y, b * tout * od,
                                   [[nt * od, cl], [1, nt * od]]),
                            in_=AP(sb, b * nchunk * free,
                                   [[free, cl], [stride * d, nt], [1, od]]),
                        ).then_inc(ssem, 16)
                    scalar.dma_start(
                        out=AP(y, cl * nt * od,
                               [[tout * od, bpc], [1, nt_last * od]]),
                        in_=AP(sb, cl * free,
                               [[nchunk * free, bpc], [stride * d, nt_last],
                                [1, od]]),
                    ).then_inc(ssem, 16)
                    scalar.wait_ge(ssem, (bpc + 1) * 16 * (r + 1))
        elif mode == "storecontig":  # diagnostic: pure store BW, 128 descs
            npart = bpc * nchunk
            @block.scalar
            def _(scalar):
                for r in range(reps):
                    for k in range(2):   # 2 x 10.7 MB ~ output size
                        scalar.dma_start(
                            out=AP(y, k * npart * free,
                                   [[free, npart], [1, free]]),
                            in_=AP(sb, 0, [[free, npart], [1, free]]),
                        ).then_inc(ssem, 16)
                    scalar.wait_ge(ssem, 2 * 16 * (r + 1))
        elif mode == "loadcontig":   # diagnostic: pure load BW, 128 descs
            npart = bpc * nchunk
            per = (bpc * t * d) // npart
            assert per <= free
            @block.sync
            def _(sync):
                for r in range(reps):
                    sync.dma_start(
                        out=AP(sb, 0, [[free, npart], [1, per]]),
                        in_=AP(x, 0, [[per, npart], [1, per]]),
                    ).then_inc(psem, 16)
                    sync.wait_ge(psem, 16 * (r + 1))
        else:
            raise ValueError(mode)

    return nc


_NC = None


def _get_nc():
    global _NC
    if _NC is None:
        _NC = build_nc()
    return _NC


def kernel(**inputs):
    x = np.ascontiguousarray(inputs["x"], dtype=np.float32)
    assert x.shape == (B, T, D)
    nc = _get_nc()
    in_maps = [{"x": x[i * BPC:(i + 1) * BPC]} for i in range(NCORES)]
    res = run_bass_kernel_spmd(nc, in_maps, list(range(NCORES)))
    return np.concatenate([res.results[i]["y"] for i in range(NCORES)], axis=0)
